# revision 15
# baseline (speedup 1.0000x reference)
"""Trainium2 Bass kernel for nn_MultiHeadedAttention_88021059764737.

Math (reference):
  q = Wq @ query + bq   (per batch; [D, N]), same k, v
  per head h (DIM=64): scores = q_h^T k_h / 8   [N, N]
  adj = dense adjacency counts from edges        [N, N]
  prob = exp(scores * adj) / rowsum
  x_h = v_h @ prob^T ; out = Wm @ x + bm

Device design (per core; 8 cores = 4 batches x 2 n-halves):
  - All weights pre-transposed/permuted on host to head-major layout; the
    1/sqrt(DIM) scale is folded into Wq/bq.
  - scores computed TRANSPOSED: scT[m, n] = k_h^T q_h (m on partitions) so
    the exp path can feed the PV matmul (contraction over m = partition dim)
    without any transposes.
  - delta decomposition: exp(s*adj) = 1 + adj*(exp(s)-1) (exact for
    adj in {0,1}; ~1e-4 rel error from rare duplicate edges). ACT computes
    exp(s) straight from PSUM; one DVE scalar_tensor_tensor forms
    delta = (exp(s)-1)*adjT; PV contracts v^T against delta and the "+1"
    term is restored via a per-partition vsum bias (vsum[d] = sum_m v^T).
  - rowsum over m is folded into the PV matmul by appending a ones column to
    v^T (M=65 matmuls): row 64 of each head's PSUM accumulator + N is the
    softmax denominator.
  - heavy matmuls run in float32r (full PE rate, ~5e-5 rel err); the
    normalization/broadcast path stays exact fp32.
  - final: xn = (x + vsum) * (1/rowsum) (rowsum broadcast via K=1 matmul),
    then the output projection with bias on ACT.
"""

import os
import sys

sys.path.insert(0, "/opt/trn_rl_repo")

import numpy as np

B, D, H, N, E = 4, 256, 4, 2048, 32768
DIM = D // H  # 64
NCORES = 8
NH = N // 2  # per-core n-half
SCALE = 1.0 / np.sqrt(np.float32(DIM))  # 1/8


def build_nc(N_=N, NH_=NH):
    import concourse.bass as bass  # noqa: F401
    import concourse.mybir as mybir
    import concourse.tile as tile
    from concourse import bacc

    f32 = mybir.dt.float32
    MT = N_ // 128          # m tiles of 128
    NCH = NH_ // 512        # n chunks of 512
    MG = MT // 4            # adj groups of 4 m-tiles
    Exp = mybir.ActivationFunctionType.Exp
    Ident = mybir.ActivationFunctionType.Identity
    mult = mybir.AluOpType.mult
    f32r = mybir.dt.float32r

    nc = bacc.Bacc()
    # ---- DRAM I/O ----
    xq = nc.dram_tensor("xq", [D, NH_], f32r, kind="ExternalInput")
    xk = nc.dram_tensor("xk", [D, N_], f32r, kind="ExternalInput")
    xv = nc.dram_tensor("xv", [D, N_], f32r, kind="ExternalInput")
    wq = nc.dram_tensor("wq", [128, 2, D], f32r, kind="ExternalInput")  # [p, kc, o]
    wk = nc.dram_tensor("wk", [128, 2, D], f32r, kind="ExternalInput")
    wv = nc.dram_tensor("wv", [128, 2, D], f32r, kind="ExternalInput")  # [p, kc, dd]
    wm = nc.dram_tensor("wm", [128, 2, D], f32r, kind="ExternalInput")
    bq = nc.dram_tensor("bq", [128, 2], f32, kind="ExternalInput")
    bk = nc.dram_tensor("bk", [128, 2], f32, kind="ExternalInput")
    bv = nc.dram_tensor("bv", [1, D], f32, kind="ExternalInput")
    bm = nc.dram_tensor("bm", [128, 2], f32, kind="ExternalInput")
    adjT = nc.dram_tensor("adjT", [N_, NH_], f32, kind="ExternalInput")
    out = nc.dram_tensor("out", [D, NH_], f32, kind="ExternalOutput")

    from contextlib import ExitStack

    with tile.TileContext(nc) as tc, ExitStack() as ctx:
        consts = ctx.enter_context(tc.tile_pool(name="consts", bufs=1))
        big = ctx.enter_context(tc.tile_pool(name="big", bufs=1))
        work = ctx.enter_context(tc.tile_pool(name="work", bufs=2))
        pacc = ctx.enter_context(tc.tile_pool(name="pacc", bufs=1, space="PSUM"))
        psc = ctx.enter_context(tc.tile_pool(name="psc", bufs=2, space="PSUM"))

        # ---- constants ----
        wq_sb = consts.tile([128, 2, D], f32r, tag="wq")
        wk_sb = consts.tile([128, 2, D], f32r, tag="wk")
        wv_sb = consts.tile([128, 2, D], f32r, tag="wv")
        wm_sb = consts.tile([128, 2, D], f32r, tag="wm")
        bq_sb = consts.tile([128, 2], f32, tag="bq")
        bk_sb = consts.tile([128, 2], f32, tag="bk")
        bv_sb = consts.tile([1, D], f32, tag="bv")
        bm_sb = consts.tile([128, 2], f32, tag="bm")
        ones_sb = consts.tile([1, 128], f32, tag="ones")
        for sb, dr in ((wq_sb, wq), (wk_sb, wk), (wv_sb, wv), (wm_sb, wm),
                       (bq_sb, bq), (bk_sb, bk), (bv_sb, bv), (bm_sb, bm)):
            nc.sync.dma_start(out=sb[:], in_=dr[:])
        nc.vector.memset(ones_sb[:], 1.0)

        # ---- adjacency tiles: groups of 4 m-tiles ----
        # first two groups get their own slots (loaded immediately);
        # later groups recycle the input-chunk slots after projections.
        adj_g = [None] * MG
        n_early = min(2, MG)
        for g in range(n_early):
            adj_g[g] = big.tile([128, 4, NH_], f32, tag=f"adjE{g}", name=f"adjE{g}")
            nc.sync.dma_start(
                out=adj_g[g][:],
                in_=adjT[512 * g:512 * (g + 1), :].rearrange(
                    "(mt p) n -> p mt n", p=128),
            )

        # ---- load inputs ----
        xq_sb = big.tile([128, 2, NH_], f32r, tag="kv0")
        xk_sb = big.tile([128, 2, N_], f32r, tag="kv1")
        xv_sb = big.tile([128, 2, N_], f32r, tag="kv2")
        for kc in range(2):
            nc.sync.dma_start(out=xq_sb[:, kc, :], in_=xq[128 * kc:128 * (kc + 1), :])
            nc.sync.dma_start(out=xk_sb[:, kc, :], in_=xk[128 * kc:128 * (kc + 1), :])
            nc.sync.dma_start(out=xv_sb[:, kc, :], in_=xv[128 * kc:128 * (kc + 1), :])

        # ---- projections ----
        q_sb = big.tile([128, 2, NH_], f32r, tag="q")     # [2 heads of pair, pair, n]
        k_sb = big.tile([128, 2, N_], f32r, tag="k")
        vT_sb = big.tile([128, MT, 4 * 65], f32r, tag="vT")  # per head: 64 v + 1 ones

        for p in range(2):
            for ncol in range(NH_ // 512):
                ps = psc.tile([128, 1024], f32, tag="sc")
                for kc in range(2):
                    nc.tensor.matmul(
                        ps[:, 0:512],
                        lhsT=wq_sb[:, kc, 128 * p:128 * (p + 1)],
                        rhs=xq_sb[:, kc, 512 * ncol:512 * (ncol + 1)],
                        start=(kc == 0), stop=(kc == 1),
                    )
                nc.scalar.activation(
                    out=q_sb[:, p, 512 * ncol:512 * (ncol + 1)], in_=ps[:, 0:512],
                    func=Ident, bias=bq_sb[:, p:p + 1], scale=1.0,
                )
        for p in range(2):
            for ncol in range(N_ // 512):
                ps = psc.tile([128, 1024], f32, tag="sc")
                for kc in range(2):
                    nc.tensor.matmul(
                        ps[:, 0:512],
                        lhsT=wk_sb[:, kc, 128 * p:128 * (p + 1)],
                        rhs=xk_sb[:, kc, 512 * ncol:512 * (ncol + 1)],
                        start=(kc == 0), stop=(kc == 1),
                    )
                nc.scalar.activation(
                    out=k_sb[:, p, 512 * ncol:512 * (ncol + 1)], in_=ps[:, 0:512],
                    func=Ident, bias=bk_sb[:, p:p + 1], scale=1.0,
                )
        # ones columns for the rowsum trick
        nc.vector.memset(
            vT_sb.rearrange("q mt (h e) -> q mt h e", e=65)[:, :, :, 64:65]
            .bitcast(f32), 1.0)
        for mt in range(MT):
            ps = psc.tile([128, 1024], f32, tag="sc")
            for kc in range(2):
                nc.tensor.matmul(
                    ps[:, 0:256],
                    lhsT=xv_sb[:, kc, 128 * mt:128 * (mt + 1)],
                    rhs=wv_sb[:, kc, :],
                    start=(kc == 0), stop=False,
                )
            nc.tensor.matmul(  # + bias via K=1 ones row
                ps[:, 0:256], lhsT=ones_sb[0:1, 0:128], rhs=bv_sb[0:1, :],
                start=False, stop=True,
            )
            nc.vector.tensor_copy(
                out=vT_sb.rearrange("q mt (h e) -> q mt h e", e=65)[:, mt, :, 0:64],
                in_=ps[:, 0:256].rearrange("q (h d) -> q h d", d=64),
            )

        # ---- vsum[d] = sum_m vT[m, d] (row 64 gives N = full rowsum base) ----
        onescol = consts.tile([128, 1], f32, tag="onescol")
        nc.vector.memset(onescol[:], 1.0)
        vs_ps = psc.tile([128, 1024], f32, tag="sc", name="vs_ps")
        for hh in range(4):
            for mt in range(MT):
                nc.tensor.matmul(
                    vs_ps[0:65, hh:hh + 1],
                    lhsT=vT_sb[:, mt, 65 * hh:65 * (hh + 1)].bitcast(f32),
                    rhs=onescol[:],
                    start=(mt == 0), stop=(mt == MT - 1),
                )
        vsum_sb = consts.tile([128, 4], f32, tag="vsum")
        zrow_sb = consts.tile([1, 4], f32, tag="zrow")
        nc.vector.tensor_copy(out=vsum_sb[0:64, :], in_=vs_ps[0:64, 0:4])
        nc.vector.tensor_copy(out=vsum_sb[64:128, :], in_=vs_ps[0:64, 0:4])
        nc.vector.tensor_copy(out=zrow_sb[:], in_=vs_ps[64:65, 0:4])

        # ---- late adj groups (recycle input slots) ----
        for g in range(n_early, MG):
            tagn = ("kv0", "kv1", "kv2")[(g - n_early) % 3] if (g - n_early) < 3 \
                else f"adjL{g}"
            adj_g[g] = big.tile([128, 4, NH_], f32, tag=tagn, name=f"adjL{g}")
            nc.sync.dma_start(
                out=adj_g[g][:],
                in_=adjT[512 * g:512 * (g + 1), :].rearrange(
                    "(mt p) n -> p mt n", p=128),
            )

        # ---- attention ----
        for c in range(NCH):
            nsl = slice(512 * c, 512 * (c + 1))
            x_h = [pacc.tile([128, 512], f32, tag=f"x{hh}", name=f"x{hh}") for hh in range(4)]
            for mt in range(MT):
                adjtile = adj_g[mt // 4]
                mtl = mt % 4
                adj_sl = adjtile[:, mtl, nsl]
                adj_b = bass.AP(tensor=adj_sl.tensor, offset=adj_sl.offset,
                                ap=[adj_sl.ap[0], [0, 2], adj_sl.ap[-1]])
                d_t = work.tile([128, 4, 512], f32r, tag="dlt")
                for p in range(2):
                    scp = psc.tile([128, 1024], f32, tag="sc", name="scp")
                    for h in range(2):
                        nc.tensor.matmul(
                            scp[:, 512 * h:512 * (h + 1)],
                            lhsT=k_sb[64 * h:64 * (h + 1), p, 128 * mt:128 * (mt + 1)],
                            rhs=q_sb[64 * h:64 * (h + 1), p, nsl],
                            start=True, stop=True,
                        )
                    e_t = work.tile([128, 1024], f32, tag="exp", name="e_t")
                    nc.scalar.activation(out=e_t[:], in_=scp[:], func=Exp)
                    # delta = (exp(s) - 1) * adj   (exact for adj in {0,1})
                    nc.vector.scalar_tensor_tensor(
                        out=d_t[:, 2 * p:2 * p + 2, :],
                        in0=e_t[:].rearrange("q (h n) -> q h n", h=2),
                        scalar=-1.0,
                        in1=adj_b,
                        op0=mybir.AluOpType.add,
                        op1=mult,
                    )
                for hh in range(4):
                    nc.tensor.matmul(
                        x_h[hh][0:65, :],
                        lhsT=vT_sb[:, mt, 65 * hh:65 * (hh + 1)],
                        rhs=d_t[:, hh, :],
                        start=(mt == 0), stop=(mt == MT - 1),
                    )

            # ---- normalize ----
            recip_t = [work.tile([1, 512], f32, tag=f"rcp{hh}", name=f"rcp{hh}")
                       for hh in range(4)]
            for hh in range(4):
                nc.vector.tensor_scalar_add(
                    recip_t[hh][:], x_h[hh][64:65, :], zrow_sb[0:1, hh:hh + 1])
                nc.vector.reciprocal(out=recip_t[hh][:], in_=recip_t[hh][:])
            zb_ps = psc.tile([128, 1024], f32, tag="sc")
            for hh in range(4):
                h, p = hh % 2, hh // 2
                nc.tensor.matmul(
                    zb_ps[64 * h:64 * (h + 1), 512 * p:512 * (p + 1)],
                    lhsT=ones_sb[0:1, 0:64],
                    rhs=recip_t[hh][:],
                    start=True, stop=True,
                )
            zb_sb = work.tile([128, 1024], f32, tag="zbs")
            nc.vector.tensor_copy(out=zb_sb[:], in_=zb_ps[:])
            xn_sb = work.tile([128, 2, 512], f32r, tag="xn")
            for hh in range(4):
                h, p = hh % 2, hh // 2
                nc.vector.scalar_tensor_tensor(
                    out=xn_sb[64 * h:64 * (h + 1), p, :],
                    in0=x_h[hh][0:64, :],
                    scalar=vsum_sb[64 * h:64 * h + 64, hh:hh + 1],
                    in1=zb_sb[64 * h:64 * (h + 1), 512 * p:512 * (p + 1)],
                    op0=mybir.AluOpType.add,
                    op1=mult,
                )
            # ---- output projection ----
            for mtile in range(2):
                op_ps = psc.tile([128, 1024], f32, tag="sc")
                for kc in range(2):
                    nc.tensor.matmul(
                        op_ps[:, 0:512],
                        lhsT=wm_sb[:, kc, 128 * mtile:128 * (mtile + 1)],
                        rhs=xn_sb[:, kc, :],
                        start=(kc == 0), stop=(kc == 1),
                    )
                out_t = work.tile([128, 512], f32, tag="osb")
                nc.scalar.activation(
                    out=out_t[:], in_=op_ps[:, 0:512],
                    func=Ident, bias=bm_sb[:, mtile:mtile + 1], scale=1.0,
                )
                nc.sync.dma_start(
                    out=out[128 * mtile:128 * (mtile + 1), nsl], in_=out_t[:])

    nc.compile()
    return nc


def host_prep(query, key, value, edges, Wq, bq, Wk, bk, Wv, bv, Wm, bm,
              N_=N, NH_=NH, B_=B):
    """Returns per-core input maps."""
    f32 = np.float32
    query = np.asarray(query, f32)
    key = np.asarray(key, f32)
    value = np.asarray(value, f32)
    edges = np.asarray(edges)
    Wq, bq = np.asarray(Wq, f32), np.asarray(bq, f32)
    Wk, bk = np.asarray(Wk, f32), np.asarray(bk, f32)
    Wv, bv = np.asarray(Wv, f32), np.asarray(bv, f32)
    Wm, bm = np.asarray(Wm, f32), np.asarray(bm, f32)

    # head-major permutation: dd = h*DIM + dl  <->  o = dl*H + h
    dd = np.arange(D)
    perm = (dd % DIM) * H + (dd // DIM)

    def lhsT_layout(WT):  # WT [256(K), 256(M)] -> [128, 2, 256]
        return np.ascontiguousarray(WT.reshape(2, 128, D).transpose(1, 0, 2))

    wq_dev = lhsT_layout((Wq[perm, :] * SCALE).T)
    wk_dev = lhsT_layout(Wk[perm, :].T)
    wv_dev = lhsT_layout(Wv[perm, :].T)      # rhs[d_in, dd]: Wv_perm.T
    wm_dev = lhsT_layout(Wm[:, perm].T)      # lhsT[dd, o]
    bq_dev = np.ascontiguousarray((bq[perm] * SCALE).reshape(2, 128).T)
    bk_dev = np.ascontiguousarray(bk[perm].reshape(2, 128).T)
    bv_dev = np.ascontiguousarray(bv[perm].reshape(1, D))
    bm_dev = np.ascontiguousarray(bm.reshape(2, 128).T)

    in_maps = []
    ncores = 2 * B_
    for c in range(ncores):
        b, half = c // 2, c % 2
        ns = slice(half * NH_, (half + 1) * NH_)
        adj = np.zeros((N_, N_), f32)
        np.add.at(adj, (edges[b, 0].astype(np.int64),
                        edges[b, 1].astype(np.int64)), 1.0)
        adjT_c = np.ascontiguousarray(adj[ns, :].T)
        in_maps.append({
            "xq": np.ascontiguousarray(query[b][:, ns]),
            "xk": np.ascontiguousarray(key[b]),
            "xv": np.ascontiguousarray(value[b]),
            "wq": wq_dev, "wk": wk_dev, "wv": wv_dev, "wm": wm_dev,
            "bq": bq_dev, "bk": bk_dev, "bv": bv_dev, "bm": bm_dev,
            "adjT": adjT_c,
        })
    return in_maps


LAST_RESULTS = None
LAST_NC = None


def kernel(**inputs):
    global LAST_RESULTS, LAST_NC
    from concourse.bass_utils import run_bass_kernel_spmd

    in_maps = host_prep(**inputs)
    nc = build_nc()
    LAST_NC = nc
    trace = bool(int(os.environ.get("KERNEL_TRACE", "0")))
    res = run_bass_kernel_spmd(nc, in_maps, core_ids=list(range(NCORES)),
                               trace=trace)
    LAST_RESULTS = res
    out = np.empty((B, D, N), np.float32)
    for c in range(NCORES):
        b, half = c // 2, c % 2
        out[b][:, half * NH:(half + 1) * NH] = res.results[c]["out"]
    return out


# revision 16
# speedup vs baseline: 1.0592x; 1.0592x over previous
"""Trainium2 Bass kernel for nn_MultiHeadedAttention_88021059764737.

Math (reference):
  q = Wq @ query + bq   (per batch; [D, N]), same k, v
  per head h (DIM=64): scores = q_h^T k_h / 8   [N, N]
  adj = dense adjacency counts from edges        [N, N]
  prob = exp(scores * adj) / rowsum
  x_h = v_h @ prob^T ; out = Wm @ x + bm

Device design (per core; 8 cores = 4 batches x 2 n-halves):
  - All weights pre-transposed/permuted on host to head-major layout; the
    1/sqrt(DIM) scale is folded into Wq/bq.
  - scores computed TRANSPOSED: scT[m, n] = k_h^T q_h (m on partitions) so
    the exp path can feed the PV matmul (contraction over m = partition dim)
    without any transposes.
  - delta decomposition: exp(s*adj) = 1 + adj*(exp(s)-1) (exact for
    adj in {0,1}; ~1e-4 rel error from rare duplicate edges). ACT computes
    exp(s) straight from PSUM; one DVE scalar_tensor_tensor forms
    delta = (exp(s)-1)*adjT; PV contracts v^T against delta and the "+1"
    term is restored via a per-partition vsum bias (vsum[d] = sum_m v^T).
  - rowsum over m is folded into the PV matmul by appending a ones column to
    v^T (M=65 matmuls): row 64 of each head's PSUM accumulator + N is the
    softmax denominator.
  - heavy matmuls run in float32r (full PE rate, ~5e-5 rel err); the
    normalization/broadcast path stays exact fp32.
  - final: xn = (x + vsum) * (1/rowsum) (rowsum broadcast via K=1 matmul),
    then the output projection with bias on ACT.
"""

import os
import sys

sys.path.insert(0, "/opt/trn_rl_repo")

import numpy as np

B, D, H, N, E = 4, 256, 4, 2048, 32768
DIM = D // H  # 64
NCORES = 8
NH = N // 2  # per-core n-half
SCALE = 1.0 / np.sqrt(np.float32(DIM))  # 1/8


def build_nc(N_=N, NH_=NH):
    import concourse.bass as bass  # noqa: F401
    import concourse.mybir as mybir
    import concourse.tile as tile
    from concourse import bacc

    f32 = mybir.dt.float32
    MT = N_ // 128          # m tiles of 128
    NCH = NH_ // 512        # n chunks of 512
    MG = MT // 4            # adj groups of 4 m-tiles
    Exp = mybir.ActivationFunctionType.Exp
    Ident = mybir.ActivationFunctionType.Identity
    mult = mybir.AluOpType.mult
    f32r = mybir.dt.float32r

    nc = bacc.Bacc()
    # ---- DRAM I/O ----
    xq = nc.dram_tensor("xq", [D, NH_], f32r, kind="ExternalInput")
    xk = nc.dram_tensor("xk", [D, N_], f32r, kind="ExternalInput")
    xv = nc.dram_tensor("xv", [D, N_], f32r, kind="ExternalInput")
    wq = nc.dram_tensor("wq", [128, 2, D], f32r, kind="ExternalInput")  # [p, kc, o]
    wk = nc.dram_tensor("wk", [128, 2, D], f32r, kind="ExternalInput")
    wv = nc.dram_tensor("wv", [128, 2, D], f32r, kind="ExternalInput")  # [p, kc, dd]
    wm = nc.dram_tensor("wm", [128, 2, D], f32r, kind="ExternalInput")
    bq = nc.dram_tensor("bq", [128, 2], f32, kind="ExternalInput")
    bk = nc.dram_tensor("bk", [128, 2], f32, kind="ExternalInput")
    bv = nc.dram_tensor("bv", [1, D], f32, kind="ExternalInput")
    bm = nc.dram_tensor("bm", [128, 2], f32, kind="ExternalInput")
    adjT = nc.dram_tensor("adjT", [N_, NH_], f32, kind="ExternalInput")
    out = nc.dram_tensor("out", [D, NH_], f32, kind="ExternalOutput")

    from contextlib import ExitStack

    with tile.TileContext(nc) as tc, ExitStack() as ctx:
        consts = ctx.enter_context(tc.tile_pool(name="consts", bufs=1))
        big = ctx.enter_context(tc.tile_pool(name="big", bufs=1))
        work = ctx.enter_context(tc.tile_pool(name="work", bufs=2))
        pacc = ctx.enter_context(tc.tile_pool(name="pacc", bufs=1, space="PSUM"))
        psc = ctx.enter_context(tc.tile_pool(name="psc", bufs=2, space="PSUM"))

        # ---- constants ----
        wq_sb = consts.tile([128, 2, D], f32r, tag="wq")
        wk_sb = consts.tile([128, 2, D], f32r, tag="wk")
        wv_sb = consts.tile([128, 2, D], f32r, tag="wv")
        wm_sb = consts.tile([128, 2, D], f32r, tag="wm")
        bq_sb = consts.tile([128, 2], f32, tag="bq")
        bk_sb = consts.tile([128, 2], f32, tag="bk")
        bv_sb = consts.tile([1, D], f32, tag="bv")
        bm_sb = consts.tile([128, 2], f32, tag="bm")
        ones_sb = consts.tile([1, 128], f32, tag="ones")
        for sb, dr in ((wq_sb, wq), (wk_sb, wk), (wv_sb, wv), (wm_sb, wm),
                       (bq_sb, bq), (bk_sb, bk), (bv_sb, bv), (bm_sb, bm)):
            nc.sync.dma_start(out=sb[:], in_=dr[:])
        nc.vector.memset(ones_sb[:], 1.0)

        # ---- adjacency tiles: groups of 4 m-tiles ----
        # first two groups get their own slots (loaded immediately);
        # later groups recycle the input-chunk slots after projections.
        adj_g = [None] * MG
        n_early = min(2, MG)
        for g in range(n_early):
            adj_g[g] = big.tile([128, 4, NH_], f32, tag=f"adjE{g}", name=f"adjE{g}")
            nc.sync.dma_start(
                out=adj_g[g][:],
                in_=adjT[512 * g:512 * (g + 1), :].rearrange(
                    "(mt p) n -> p mt n", p=128),
            )

        # ---- load inputs ----
        xq_sb = big.tile([128, 2, NH_], f32r, tag="kv0")
        xk_sb = big.tile([128, 2, N_], f32r, tag="kv1")
        xv_sb = big.tile([128, 2, N_], f32r, tag="kv2")
        for kc in range(2):
            nc.sync.dma_start(out=xq_sb[:, kc, :], in_=xq[128 * kc:128 * (kc + 1), :])
            nc.sync.dma_start(out=xk_sb[:, kc, :], in_=xk[128 * kc:128 * (kc + 1), :])
            nc.sync.dma_start(out=xv_sb[:, kc, :], in_=xv[128 * kc:128 * (kc + 1), :])

        # ---- projections ----
        q_sb = big.tile([128, 2, NH_], f32r, tag="q")     # [2 heads of pair, pair, n]
        k_sb = big.tile([128, 2, N_], f32r, tag="k")
        vT_sb = big.tile([128, MT, 4 * 65], f32r, tag="vT")  # per head: 64 v + 1 ones

        for p in range(2):
            for ncol in range(NH_ // 512):
                ps = psc.tile([128, 1024], f32, tag="sc")
                for kc in range(2):
                    nc.tensor.matmul(
                        ps[:, 0:512],
                        lhsT=wq_sb[:, kc, 128 * p:128 * (p + 1)],
                        rhs=xq_sb[:, kc, 512 * ncol:512 * (ncol + 1)],
                        start=(kc == 0), stop=(kc == 1),
                    )
                nc.scalar.activation(
                    out=q_sb[:, p, 512 * ncol:512 * (ncol + 1)], in_=ps[:, 0:512],
                    func=Ident, bias=bq_sb[:, p:p + 1], scale=1.0,
                )
        for p in range(2):
            for ncol in range(N_ // 512):
                ps = psc.tile([128, 1024], f32, tag="sc")
                for kc in range(2):
                    nc.tensor.matmul(
                        ps[:, 0:512],
                        lhsT=wk_sb[:, kc, 128 * p:128 * (p + 1)],
                        rhs=xk_sb[:, kc, 512 * ncol:512 * (ncol + 1)],
                        start=(kc == 0), stop=(kc == 1),
                    )
                nc.scalar.activation(
                    out=k_sb[:, p, 512 * ncol:512 * (ncol + 1)], in_=ps[:, 0:512],
                    func=Ident, bias=bk_sb[:, p:p + 1], scale=1.0,
                )
        # ones columns for the rowsum trick
        nc.vector.memset(
            vT_sb.rearrange("q mt (h e) -> q mt h e", e=65)[:, :, :, 64:65]
            .bitcast(f32), 1.0)
        for mt in range(MT):
            ps = psc.tile([128, 1024], f32, tag="sc")
            for kc in range(2):
                nc.tensor.matmul(
                    ps[:, 0:256],
                    lhsT=xv_sb[:, kc, 128 * mt:128 * (mt + 1)],
                    rhs=wv_sb[:, kc, :],
                    start=(kc == 0), stop=False,
                )
            nc.tensor.matmul(  # + bias via K=1 ones row
                ps[:, 0:256], lhsT=ones_sb[0:1, 0:128], rhs=bv_sb[0:1, :],
                start=False, stop=True,
            )
            nc.vector.tensor_copy(
                out=vT_sb.rearrange("q mt (h e) -> q mt h e", e=65)[:, mt, :, 0:64],
                in_=ps[:, 0:256].rearrange("q (h d) -> q h d", d=64),
            )

        # ---- vsum[d] = sum_m vT[m, d] (row 64 gives N = full rowsum base) ----
        onescol = consts.tile([128, 1], f32, tag="onescol")
        nc.vector.memset(onescol[:], 1.0)
        vs_ps = psc.tile([128, 1024], f32, tag="sc", name="vs_ps")
        for hh in range(4):
            for mt in range(MT):
                nc.tensor.matmul(
                    vs_ps[0:65, hh:hh + 1],
                    lhsT=vT_sb[:, mt, 65 * hh:65 * (hh + 1)].bitcast(f32),
                    rhs=onescol[:],
                    start=(mt == 0), stop=(mt == MT - 1),
                )
        vsum_sb = consts.tile([128, 4], f32, tag="vsum")
        zrow_sb = consts.tile([1, 4], f32, tag="zrow")
        nc.vector.tensor_copy(out=vsum_sb[0:64, :], in_=vs_ps[0:64, 0:4])
        nc.vector.tensor_copy(out=vsum_sb[64:128, :], in_=vs_ps[0:64, 0:4])
        nc.vector.tensor_copy(out=zrow_sb[:], in_=vs_ps[64:65, 0:4])

        # ---- late adj groups (recycle input slots) ----
        for g in range(n_early, MG):
            tagn = ("kv0", "kv1", "kv2")[(g - n_early) % 3] if (g - n_early) < 3 \
                else f"adjL{g}"
            adj_g[g] = big.tile([128, 4, NH_], f32, tag=tagn, name=f"adjL{g}")
            nc.sync.dma_start(
                out=adj_g[g][:],
                in_=adjT[512 * g:512 * (g + 1), :].rearrange(
                    "(mt p) n -> p mt n", p=128),
            )

        # ---- attention ----
        for c in range(NCH):
            nsl = slice(512 * c, 512 * (c + 1))
            x_h = [pacc.tile([128, 512], f32, tag=f"x{hh}", name=f"x{hh}") for hh in range(4)]
            for mt in range(MT):
                adjtile = adj_g[mt // 4]
                mtl = mt % 4
                adj_sl = adjtile[:, mtl, nsl]
                adj_b = bass.AP(tensor=adj_sl.tensor, offset=adj_sl.offset,
                                ap=[adj_sl.ap[0], [0, 2], adj_sl.ap[-1]])
                d_t = work.tile([128, 4, 512], f32r, tag="dlt")
                for p in range(2):
                    scp = psc.tile([128, 1024], f32, tag="sc", name="scp")
                    for h in range(2):
                        nc.tensor.matmul(
                            scp[:, 512 * h:512 * (h + 1)],
                            lhsT=k_sb[64 * h:64 * (h + 1), p, 128 * mt:128 * (mt + 1)],
                            rhs=q_sb[64 * h:64 * (h + 1), p, nsl],
                            start=True, stop=True,
                        )
                    e_t = work.tile([128, 1024], f32, tag="exp", name="e_t", bufs=4)
                    nc.scalar.activation(out=e_t[:], in_=scp[:], func=Exp)
                    # delta = (exp(s) - 1) * adj   (exact for adj in {0,1})
                    nc.vector.scalar_tensor_tensor(
                        out=d_t[:, 2 * p:2 * p + 2, :],
                        in0=e_t[:].rearrange("q (h n) -> q h n", h=2),
                        scalar=-1.0,
                        in1=adj_b,
                        op0=mybir.AluOpType.add,
                        op1=mult,
                    )
                for hh in range(4):
                    nc.tensor.matmul(
                        x_h[hh][0:65, :],
                        lhsT=vT_sb[:, mt, 65 * hh:65 * (hh + 1)],
                        rhs=d_t[:, hh, :],
                        start=(mt == 0), stop=(mt == MT - 1),
                    )

            # ---- normalize ----
            recip_t = [work.tile([1, 512], f32, tag=f"rcp{hh}", name=f"rcp{hh}")
                       for hh in range(4)]
            for hh in range(4):
                nc.vector.tensor_scalar_add(
                    recip_t[hh][:], x_h[hh][64:65, :], zrow_sb[0:1, hh:hh + 1])
                nc.vector.reciprocal(out=recip_t[hh][:], in_=recip_t[hh][:])
            zb_ps = psc.tile([128, 1024], f32, tag="sc")
            for hh in range(4):
                h, p = hh % 2, hh // 2
                nc.tensor.matmul(
                    zb_ps[64 * h:64 * (h + 1), 512 * p:512 * (p + 1)],
                    lhsT=ones_sb[0:1, 0:64],
                    rhs=recip_t[hh][:],
                    start=True, stop=True,
                )
            zb_sb = work.tile([128, 1024], f32, tag="zbs")
            nc.vector.tensor_copy(out=zb_sb[:], in_=zb_ps[:])
            xn_sb = work.tile([128, 2, 512], f32r, tag="xn")
            for hh in range(4):
                h, p = hh % 2, hh // 2
                nc.vector.scalar_tensor_tensor(
                    out=xn_sb[64 * h:64 * (h + 1), p, :],
                    in0=x_h[hh][0:64, :],
                    scalar=vsum_sb[64 * h:64 * h + 64, hh:hh + 1],
                    in1=zb_sb[64 * h:64 * (h + 1), 512 * p:512 * (p + 1)],
                    op0=mybir.AluOpType.add,
                    op1=mult,
                )
            # ---- output projection ----
            for mtile in range(2):
                op_ps = psc.tile([128, 1024], f32, tag="sc")
                for kc in range(2):
                    nc.tensor.matmul(
                        op_ps[:, 0:512],
                        lhsT=wm_sb[:, kc, 128 * mtile:128 * (mtile + 1)],
                        rhs=xn_sb[:, kc, :],
                        start=(kc == 0), stop=(kc == 1),
                    )
                out_t = work.tile([128, 512], f32, tag="osb")
                nc.scalar.activation(
                    out=out_t[:], in_=op_ps[:, 0:512],
                    func=Ident, bias=bm_sb[:, mtile:mtile + 1], scale=1.0,
                )
                nc.sync.dma_start(
                    out=out[128 * mtile:128 * (mtile + 1), nsl], in_=out_t[:])

    nc.compile()
    return nc


def host_prep(query, key, value, edges, Wq, bq, Wk, bk, Wv, bv, Wm, bm,
              N_=N, NH_=NH, B_=B):
    """Returns per-core input maps."""
    f32 = np.float32
    query = np.asarray(query, f32)
    key = np.asarray(key, f32)
    value = np.asarray(value, f32)
    edges = np.asarray(edges)
    Wq, bq = np.asarray(Wq, f32), np.asarray(bq, f32)
    Wk, bk = np.asarray(Wk, f32), np.asarray(bk, f32)
    Wv, bv = np.asarray(Wv, f32), np.asarray(bv, f32)
    Wm, bm = np.asarray(Wm, f32), np.asarray(bm, f32)

    # head-major permutation: dd = h*DIM + dl  <->  o = dl*H + h
    dd = np.arange(D)
    perm = (dd % DIM) * H + (dd // DIM)

    def lhsT_layout(WT):  # WT [256(K), 256(M)] -> [128, 2, 256]
        return np.ascontiguousarray(WT.reshape(2, 128, D).transpose(1, 0, 2))

    wq_dev = lhsT_layout((Wq[perm, :] * SCALE).T)
    wk_dev = lhsT_layout(Wk[perm, :].T)
    wv_dev = lhsT_layout(Wv[perm, :].T)      # rhs[d_in, dd]: Wv_perm.T
    wm_dev = lhsT_layout(Wm[:, perm].T)      # lhsT[dd, o]
    bq_dev = np.ascontiguousarray((bq[perm] * SCALE).reshape(2, 128).T)
    bk_dev = np.ascontiguousarray(bk[perm].reshape(2, 128).T)
    bv_dev = np.ascontiguousarray(bv[perm].reshape(1, D))
    bm_dev = np.ascontiguousarray(bm.reshape(2, 128).T)

    in_maps = []
    ncores = 2 * B_
    for c in range(ncores):
        b, half = c // 2, c % 2
        ns = slice(half * NH_, (half + 1) * NH_)
        adj = np.zeros((N_, N_), f32)
        np.add.at(adj, (edges[b, 0].astype(np.int64),
                        edges[b, 1].astype(np.int64)), 1.0)
        adjT_c = np.ascontiguousarray(adj[ns, :].T)
        in_maps.append({
            "xq": np.ascontiguousarray(query[b][:, ns]),
            "xk": np.ascontiguousarray(key[b]),
            "xv": np.ascontiguousarray(value[b]),
            "wq": wq_dev, "wk": wk_dev, "wv": wv_dev, "wm": wm_dev,
            "bq": bq_dev, "bk": bk_dev, "bv": bv_dev, "bm": bm_dev,
            "adjT": adjT_c,
        })
    return in_maps


LAST_RESULTS = None
LAST_NC = None


def kernel(**inputs):
    global LAST_RESULTS, LAST_NC
    from concourse.bass_utils import run_bass_kernel_spmd

    in_maps = host_prep(**inputs)
    nc = build_nc()
    LAST_NC = nc
    trace = bool(int(os.environ.get("KERNEL_TRACE", "0")))
    res = run_bass_kernel_spmd(nc, in_maps, core_ids=list(range(NCORES)),
                               trace=trace)
    LAST_RESULTS = res
    out = np.empty((B, D, N), np.float32)
    for c in range(NCORES):
        b, half = c // 2, c % 2
        out[b][:, half * NH:(half + 1) * NH] = res.results[c]["out"]
    return out


# revision 18
# speedup vs baseline: 1.1026x; 1.0409x over previous
"""Trainium2 Bass kernel for nn_MultiHeadedAttention_88021059764737.

Math (reference):
  q = Wq @ query + bq   (per batch; [D, N]), same k, v
  per head h (DIM=64): scores = q_h^T k_h / 8   [N, N]
  adj = dense adjacency counts from edges        [N, N]
  prob = exp(scores * adj) / rowsum
  x_h = v_h @ prob^T ; out = Wm @ x + bm

Device design (per core; 8 cores = 4 batches x 2 n-halves):
  - All weights pre-transposed/permuted on host to head-major layout; the
    1/sqrt(DIM) scale is folded into Wq/bq.
  - scores computed TRANSPOSED: scT[m, n] = k_h^T q_h (m on partitions) so
    the exp path can feed the PV matmul (contraction over m = partition dim)
    without any transposes.
  - delta decomposition: exp(s*adj) = 1 + adj*(exp(s)-1) (exact for
    adj in {0,1}; ~1e-4 rel error from rare duplicate edges). ACT computes
    exp(s) straight from PSUM; one DVE scalar_tensor_tensor forms
    delta = (exp(s)-1)*adjT; PV contracts v^T against delta and the "+1"
    term is restored via a per-partition vsum bias (vsum[d] = sum_m v^T).
  - rowsum over m is folded into the PV matmul by appending a ones column to
    v^T (M=65 matmuls): row 64 of each head's PSUM accumulator + N is the
    softmax denominator.
  - heavy matmuls run in float32r (full PE rate, ~5e-5 rel err); the
    normalization/broadcast path stays exact fp32.
  - final: xn = (x + vsum) * (1/rowsum) (rowsum broadcast via K=1 matmul),
    then the output projection with bias on ACT.
"""

import os
import sys

sys.path.insert(0, "/opt/trn_rl_repo")

import numpy as np
import ml_dtypes

B, D, H, N, E = 4, 256, 4, 2048, 32768
DIM = D // H  # 64
NCORES = 8
NH = N // 2  # per-core n-half
SCALE = 1.0 / np.sqrt(np.float32(DIM))  # 1/8


def build_nc(N_=N, NH_=NH):
    import concourse.bass as bass  # noqa: F401
    import concourse.mybir as mybir
    import concourse.tile as tile
    from concourse import bacc

    f32 = mybir.dt.float32
    MT = N_ // 128          # m tiles of 128
    NCH = NH_ // 512        # n chunks of 512
    MG = MT // 4            # adj groups of 4 m-tiles
    Exp = mybir.ActivationFunctionType.Exp
    Ident = mybir.ActivationFunctionType.Identity
    mult = mybir.AluOpType.mult
    f32r = mybir.dt.float32r
    bf16 = mybir.dt.bfloat16

    nc = bacc.Bacc()
    # ---- DRAM I/O ----
    xq = nc.dram_tensor("xq", [D, NH_], f32r, kind="ExternalInput")
    xk = nc.dram_tensor("xk", [D, N_], f32r, kind="ExternalInput")
    xv = nc.dram_tensor("xv", [D, N_], f32r, kind="ExternalInput")
    wq = nc.dram_tensor("wq", [128, 2, D], f32r, kind="ExternalInput")  # [p, kc, o]
    wk = nc.dram_tensor("wk", [128, 2, D], f32r, kind="ExternalInput")
    wv = nc.dram_tensor("wv", [128, 2, D], f32r, kind="ExternalInput")  # [p, kc, dd]
    wm = nc.dram_tensor("wm", [128, 2, D], f32r, kind="ExternalInput")
    bq = nc.dram_tensor("bq", [128, 2], f32, kind="ExternalInput")
    bk = nc.dram_tensor("bk", [128, 2], f32, kind="ExternalInput")
    bv = nc.dram_tensor("bv", [1, D], f32, kind="ExternalInput")
    bm = nc.dram_tensor("bm", [128, 2], f32, kind="ExternalInput")
    adjT = nc.dram_tensor("adjT", [N_, NH_], bf16, kind="ExternalInput")
    out = nc.dram_tensor("out", [D, NH_], f32, kind="ExternalOutput")

    from contextlib import ExitStack

    with tile.TileContext(nc) as tc, ExitStack() as ctx:
        consts = ctx.enter_context(tc.tile_pool(name="consts", bufs=1))
        big = ctx.enter_context(tc.tile_pool(name="big", bufs=1))
        work = ctx.enter_context(tc.tile_pool(name="work", bufs=2))
        pacc = ctx.enter_context(tc.tile_pool(name="pacc", bufs=1, space="PSUM"))
        psc = ctx.enter_context(tc.tile_pool(name="psc", bufs=2, space="PSUM"))

        # ---- constants ----
        wq_sb = consts.tile([128, 2, D], f32r, tag="wq")
        wk_sb = consts.tile([128, 2, D], f32r, tag="wk")
        wv_sb = consts.tile([128, 2, D], f32r, tag="wv")
        wm_sb = consts.tile([128, 2, D], f32r, tag="wm")
        bq_sb = consts.tile([128, 2], f32, tag="bq")
        bk_sb = consts.tile([128, 2], f32, tag="bk")
        bv_sb = consts.tile([1, D], f32, tag="bv")
        bm_sb = consts.tile([128, 2], f32, tag="bm")
        ones_sb = consts.tile([1, 128], f32, tag="ones")
        for sb, dr in ((wq_sb, wq), (wk_sb, wk), (wv_sb, wv), (wm_sb, wm),
                       (bq_sb, bq), (bk_sb, bk), (bv_sb, bv), (bm_sb, bm)):
            nc.sync.dma_start(out=sb[:], in_=dr[:])
        nc.vector.memset(ones_sb[:], 1.0)

        # ---- adjacency tiles: groups of 4 m-tiles ----
        # first two groups get their own slots (loaded immediately);
        # later groups recycle the input-chunk slots after projections.
        adj_g = [None] * MG
        n_early = min(2, MG)
        for g in range(n_early):
            adj_g[g] = big.tile([128, 4, NH_], bf16, tag=f"adjE{g}", name=f"adjE{g}")
            nc.sync.dma_start(
                out=adj_g[g][:],
                in_=adjT[512 * g:512 * (g + 1), :].rearrange(
                    "(mt p) n -> p mt n", p=128),
            )

        # ---- load inputs ----
        xq_sb = big.tile([128, 2, NH_], f32r, tag="kv0")
        xk_sb = big.tile([128, 2, N_], f32r, tag="kv1")
        xv_sb = big.tile([128, 2, N_], f32r, tag="kv2")
        for kc in range(2):
            nc.sync.dma_start(out=xq_sb[:, kc, :], in_=xq[128 * kc:128 * (kc + 1), :])
            nc.sync.dma_start(out=xk_sb[:, kc, :], in_=xk[128 * kc:128 * (kc + 1), :])
            nc.sync.dma_start(out=xv_sb[:, kc, :], in_=xv[128 * kc:128 * (kc + 1), :])

        # ---- projections ----
        q_sb = big.tile([128, 2, NH_], f32r, tag="q")     # [2 heads of pair, pair, n]
        k_sb = big.tile([128, 2, N_], f32r, tag="k")
        vT_sb = big.tile([128, MT, 4 * 65], bf16, tag="vT")  # per head: 64 v + 1 ones

        for p in range(2):
            for ncol in range(NH_ // 512):
                ps = psc.tile([128, 1024], f32, tag="sc")
                for kc in range(2):
                    nc.tensor.matmul(
                        ps[:, 0:512],
                        lhsT=wq_sb[:, kc, 128 * p:128 * (p + 1)],
                        rhs=xq_sb[:, kc, 512 * ncol:512 * (ncol + 1)],
                        start=(kc == 0), stop=(kc == 1),
                    )
                nc.scalar.activation(
                    out=q_sb[:, p, 512 * ncol:512 * (ncol + 1)], in_=ps[:, 0:512],
                    func=Ident, bias=bq_sb[:, p:p + 1], scale=1.0,
                )
        for p in range(2):
            for ncol in range(N_ // 512):
                ps = psc.tile([128, 1024], f32, tag="sc")
                for kc in range(2):
                    nc.tensor.matmul(
                        ps[:, 0:512],
                        lhsT=wk_sb[:, kc, 128 * p:128 * (p + 1)],
                        rhs=xk_sb[:, kc, 512 * ncol:512 * (ncol + 1)],
                        start=(kc == 0), stop=(kc == 1),
                    )
                nc.scalar.activation(
                    out=k_sb[:, p, 512 * ncol:512 * (ncol + 1)], in_=ps[:, 0:512],
                    func=Ident, bias=bk_sb[:, p:p + 1], scale=1.0,
                )
        # ones columns for the rowsum trick
        nc.vector.memset(
            vT_sb.rearrange("q mt (h e) -> q mt h e", e=65)[:, :, :, 64:65], 1.0)
        for mt in range(MT):
            ps = psc.tile([128, 1024], f32, tag="sc")
            for kc in range(2):
                nc.tensor.matmul(
                    ps[:, 0:256],
                    lhsT=xv_sb[:, kc, 128 * mt:128 * (mt + 1)],
                    rhs=wv_sb[:, kc, :],
                    start=(kc == 0), stop=False,
                )
            nc.tensor.matmul(  # + bias via K=1 ones row
                ps[:, 0:256], lhsT=ones_sb[0:1, 0:128], rhs=bv_sb[0:1, :],
                start=False, stop=True,
            )
            nc.vector.tensor_copy(
                out=vT_sb.rearrange("q mt (h e) -> q mt h e", e=65)[:, mt, :, 0:64],
                in_=ps[:, 0:256].rearrange("q (h d) -> q h d", d=64),
            )

        # ---- analytic vsum[dd] = Wv_perm @ (sum_m value) + N*bv (exact f32) ----
        vv_sb = consts.tile([128, 2], f32, tag="vv")
        for kc in range(2):
            nc.vector.tensor_reduce(
                out=vv_sb[:, kc:kc + 1], in_=xv_sb[:, kc, :].bitcast(f32),
                axis=mybir.AxisListType.X, op=mybir.AluOpType.add)
        nconst = consts.tile([1, 1], f32, tag="nconst")
        nc.vector.memset(nconst[:], float(N_))
        vs_ps = psc.tile([128, 1024], f32, tag="sc", name="vs_ps")
        for mtile in range(2):
            for kc in range(2):
                nc.tensor.matmul(
                    vs_ps[0:128, mtile:mtile + 1],
                    lhsT=wv_sb[:, kc, 128 * mtile:128 * (mtile + 1)].bitcast(f32),
                    rhs=vv_sb[:, kc:kc + 1],
                    start=(kc == 0), stop=False,
                )
            nc.tensor.matmul(  # + N * bv
                vs_ps[0:128, mtile:mtile + 1],
                lhsT=bv_sb[0:1, 128 * mtile:128 * (mtile + 1)],
                rhs=nconst[:],
                start=False, stop=True,
            )
        vsum_sb = consts.tile([128, 4], f32, tag="vsum")
        zrow_sb = consts.tile([1, 4], f32, tag="zrow")
        for hh in range(4):
            sl = vs_ps[64 * (hh % 2):64 * (hh % 2) + 64, hh // 2:hh // 2 + 1]
            nc.vector.tensor_copy(out=vsum_sb[0:64, hh:hh + 1], in_=sl)
            nc.vector.tensor_copy(out=vsum_sb[64:128, hh:hh + 1], in_=sl)
        nc.vector.memset(zrow_sb[:], float(N_))

        # ---- late adj groups (recycle input slots) ----
        for g in range(n_early, MG):
            tagn = ("kv0", "kv1", "kv2")[(g - n_early) % 3] if (g - n_early) < 3 \
                else f"adjL{g}"
            adj_g[g] = big.tile([128, 4, NH_], bf16, tag=tagn, name=f"adjL{g}")
            nc.sync.dma_start(
                out=adj_g[g][:],
                in_=adjT[512 * g:512 * (g + 1), :].rearrange(
                    "(mt p) n -> p mt n", p=128),
            )

        # ---- attention ----
        for c in range(NCH):
            nsl = slice(512 * c, 512 * (c + 1))
            x_h = [pacc.tile([128, 512], f32, tag=f"x{hh}", name=f"x{hh}") for hh in range(4)]
            for mt in range(MT):
                adjtile = adj_g[mt // 4]
                mtl = mt % 4
                adj_sl = adjtile[:, mtl, nsl]
                adj_b = bass.AP(tensor=adj_sl.tensor, offset=adj_sl.offset,
                                ap=[adj_sl.ap[0], [0, 2], adj_sl.ap[-1]])
                d_t = work.tile([128, 4, 512], bf16, tag="dlt")
                for p in range(2):
                    scp = psc.tile([128, 1024], f32, tag="sc", name="scp")
                    for h in range(2):
                        nc.tensor.matmul(
                            scp[:, 512 * h:512 * (h + 1)],
                            lhsT=k_sb[64 * h:64 * (h + 1), p, 128 * mt:128 * (mt + 1)],
                            rhs=q_sb[64 * h:64 * (h + 1), p, nsl],
                            start=True, stop=True,
                        )
                    e_t = work.tile([128, 1024], bf16, tag="exp", name="e_t", bufs=4)
                    nc.scalar.activation(out=e_t[:], in_=scp[:], func=Exp)
                    # delta = (exp(s) - 1) * adj   (exact for adj in {0,1})
                    nc.vector.scalar_tensor_tensor(
                        out=d_t[:, 2 * p:2 * p + 2, :],
                        in0=e_t[:].rearrange("q (h n) -> q h n", h=2),
                        scalar=-1.0,
                        in1=adj_b,
                        op0=mybir.AluOpType.add,
                        op1=mult,
                    )
                for hh in range(4):
                    nc.tensor.matmul(
                        x_h[hh][0:65, :],
                        lhsT=vT_sb[:, mt, 65 * hh:65 * (hh + 1)],
                        rhs=d_t[:, hh, :],
                        start=(mt == 0), stop=(mt == MT - 1),
                    )

            # ---- normalize ----
            recip_t = [work.tile([1, 512], f32, tag=f"rcp{hh}", name=f"rcp{hh}")
                       for hh in range(4)]
            for hh in range(4):
                nc.vector.tensor_scalar_add(
                    recip_t[hh][:], x_h[hh][64:65, :], zrow_sb[0:1, hh:hh + 1])
                nc.vector.reciprocal(out=recip_t[hh][:], in_=recip_t[hh][:])
            zb_ps = psc.tile([128, 1024], f32, tag="sc")
            for hh in range(4):
                h, p = hh % 2, hh // 2
                nc.tensor.matmul(
                    zb_ps[64 * h:64 * (h + 1), 512 * p:512 * (p + 1)],
                    lhsT=ones_sb[0:1, 0:64],
                    rhs=recip_t[hh][:],
                    start=True, stop=True,
                )
            zb_sb = work.tile([128, 1024], f32, tag="zbs")
            nc.vector.tensor_copy(out=zb_sb[:], in_=zb_ps[:])
            xn_sb = work.tile([128, 2, 512], f32r, tag="xn")
            for hh in range(4):
                h, p = hh % 2, hh // 2
                nc.vector.scalar_tensor_tensor(
                    out=xn_sb[64 * h:64 * (h + 1), p, :],
                    in0=x_h[hh][0:64, :],
                    scalar=vsum_sb[64 * h:64 * h + 64, hh:hh + 1],
                    in1=zb_sb[64 * h:64 * (h + 1), 512 * p:512 * (p + 1)],
                    op0=mybir.AluOpType.add,
                    op1=mult,
                )
            # ---- output projection ----
            for mtile in range(2):
                op_ps = psc.tile([128, 1024], f32, tag="sc")
                for kc in range(2):
                    nc.tensor.matmul(
                        op_ps[:, 0:512],
                        lhsT=wm_sb[:, kc, 128 * mtile:128 * (mtile + 1)],
                        rhs=xn_sb[:, kc, :],
                        start=(kc == 0), stop=(kc == 1),
                    )
                out_t = work.tile([128, 512], f32, tag="osb")
                nc.scalar.activation(
                    out=out_t[:], in_=op_ps[:, 0:512],
                    func=Ident, bias=bm_sb[:, mtile:mtile + 1], scale=1.0,
                )
                nc.sync.dma_start(
                    out=out[128 * mtile:128 * (mtile + 1), nsl], in_=out_t[:])

    nc.compile()
    return nc


def host_prep(query, key, value, edges, Wq, bq, Wk, bk, Wv, bv, Wm, bm,
              N_=N, NH_=NH, B_=B):
    """Returns per-core input maps."""
    f32 = np.float32
    query = np.asarray(query, f32)
    key = np.asarray(key, f32)
    value = np.asarray(value, f32)
    edges = np.asarray(edges)
    Wq, bq = np.asarray(Wq, f32), np.asarray(bq, f32)
    Wk, bk = np.asarray(Wk, f32), np.asarray(bk, f32)
    Wv, bv = np.asarray(Wv, f32), np.asarray(bv, f32)
    Wm, bm = np.asarray(Wm, f32), np.asarray(bm, f32)

    # head-major permutation: dd = h*DIM + dl  <->  o = dl*H + h
    dd = np.arange(D)
    perm = (dd % DIM) * H + (dd // DIM)

    def lhsT_layout(WT):  # WT [256(K), 256(M)] -> [128, 2, 256]
        return np.ascontiguousarray(WT.reshape(2, 128, D).transpose(1, 0, 2))

    wq_dev = lhsT_layout((Wq[perm, :] * SCALE).T)
    wk_dev = lhsT_layout(Wk[perm, :].T)
    wv_dev = lhsT_layout(Wv[perm, :].T)      # rhs[d_in, dd]: Wv_perm.T
    wm_dev = lhsT_layout(Wm[:, perm].T)      # lhsT[dd, o]
    bq_dev = np.ascontiguousarray((bq[perm] * SCALE).reshape(2, 128).T)
    bk_dev = np.ascontiguousarray(bk[perm].reshape(2, 128).T)
    bv_dev = np.ascontiguousarray(bv[perm].reshape(1, D))
    bm_dev = np.ascontiguousarray(bm.reshape(2, 128).T)

    in_maps = []
    ncores = 2 * B_
    for c in range(ncores):
        b, half = c // 2, c % 2
        ns = slice(half * NH_, (half + 1) * NH_)
        adj = np.zeros((N_, N_), f32)
        np.add.at(adj, (edges[b, 0].astype(np.int64),
                        edges[b, 1].astype(np.int64)), 1.0)
        adjT_c = np.ascontiguousarray(adj[ns, :].T).astype(ml_dtypes.bfloat16)
        in_maps.append({
            "xq": np.ascontiguousarray(query[b][:, ns]),
            "xk": np.ascontiguousarray(key[b]),
            "xv": np.ascontiguousarray(value[b]),
            "wq": wq_dev, "wk": wk_dev, "wv": wv_dev, "wm": wm_dev,
            "bq": bq_dev, "bk": bk_dev, "bv": bv_dev, "bm": bm_dev,
            "adjT": adjT_c,
        })
    return in_maps


LAST_RESULTS = None
LAST_NC = None


def kernel(**inputs):
    global LAST_RESULTS, LAST_NC
    from concourse.bass_utils import run_bass_kernel_spmd

    in_maps = host_prep(**inputs)
    nc = build_nc()
    LAST_NC = nc
    trace = bool(int(os.environ.get("KERNEL_TRACE", "0")))
    res = run_bass_kernel_spmd(nc, in_maps, core_ids=list(range(NCORES)),
                               trace=trace)
    LAST_RESULTS = res
    out = np.empty((B, D, N), np.float32)
    for c in range(NCORES):
        b, half = c // 2, c % 2
        out[b][:, half * NH:(half + 1) * NH] = res.results[c]["out"]
    return out
